# revision 2
# baseline (speedup 1.0000x reference)
"""Trainium2 Bass kernel for y = x @ W^T + b  (4096x4096 @ 4096x4096 + 4096).

Sharding: data-parallel over batch, R=8 groups. Core r computes
yT_r = W @ x_r^T + b[:, None]  ([4096, 512], output transposed) and the
host reassembles y. No collectives.

All layout work happens on the host: x and W are transposed, tiled to
the exact SBUF layout, and cast to bf16 in numpy. The device kernel is
nothing but back-to-back bf16 matmuls (fp32 PSUM accumulate).

v2 schedule (from ntff trace analysis of v1 at 248.7us):
  - x (4MB) is split across TWO queues: k0-15 on the scalar HWDGE queue
    (starts moving ~8.4us, right after the ~6.6us framework preamble),
    k16-31 on the gpsimd SWDGE ring (~11us). Combined ~210 GB/s.
  - W streams on the sync HWDGE queue (~148 GB/s steady demand). The
    first two slabs are split into k-range pieces so o-tile 0's first
    matmuls can start ~9us instead of ~22.5us.
  - A short junk-matmul burst (memset-gated, no DMA dependency) warms
    the HAM clock gate from ~7us; small junk fillers bridge x-arrival
    gaps in the prologue so the PE never idles >3.4us (re-throttle).
  - Prologue: o-tiles 0-2 run chunk-major in x-arrival order.
  - Steady state per o-tile: 1MB W slab DMA (triple-buffered), 32
    k-matmuls (N=512) into one PSUM bank, ScalarE eviction fused with
    bias add, HWDGE DMA out.
  - Tail: the last o-tile accumulates in two half-width PSUM groups so
    the final eviction+output DMA is 512KB overlapped, not 1MB serial.

PE roofline: 1024 MM x 512 cols @ 2.4 GHz + ~2.5ns/MM issue = 221.2 us.
"""

import os
import sys

for _p in ("/opt/trn_rl_repo", "/opt/pypackages"):
    if _p not in sys.path and os.path.isdir(_p):
        sys.path.append(_p)

import numpy as np
import ml_dtypes

import concourse.bass as bass
import concourse.tile as tile
from concourse import bacc, mybir
from concourse.bass_utils import run_bass_kernel_spmd

N_CORES = 8
R = 8                          # batch groups
BATCH = 4096
IN_F = 4096
OUT_F = 4096
P = 128
BR = BATCH // R                # 512 batch rows per core
KT = IN_F // P                 # 32 contraction tiles
OT = OUT_F // P                # 32 output-feature tiles per core

# x chunk schedule in k-tiles: (start, len). A-chunks ride the scalar
# HWDGE queue (fast start), B-chunks the gpsimd SWDGE ring.
XCHUNKS_A = [(0, 1), (1, 1), (2, 2), (4, 4), (8, 8)]
XCHUNKS_B = [(16, 8), (24, 8)]
# W slab pieces (k-ranges) for the first two o-tiles; later slabs are
# single DMAs.
WPIECES = {0: [(0, 4), (4, 12), (16, 16)], 1: [(0, 16), (16, 16)]}
NPRE = 3                       # o-tiles interleaved in the prologue

_F32 = mybir.dt.float32
_BF16 = mybir.dt.bfloat16
_BF16_NP = ml_dtypes.bfloat16

_compiled_nc = None


def _build():
    nc = bacc.Bacc("TRN2", target_bir_lowering=False, debug=False,
                   num_devices=N_CORES)

    # Host-pretiled layouts (see _prep_inputs):
    #   xt[p, it*BR + b]            = x_r[b, it*128 + p]            (bf16)
    #   wt[ot*128 + p, it*128 + o2] = w[ot*128 + o2, it*128 + p]    (bf16)
    #   bias_t[p, ot]               = b[ot*128 + p]                 (f32)
    xt = nc.dram_tensor("xt", [P, KT * BR], _BF16, kind="ExternalInput")
    wt = nc.dram_tensor("wt", [OT * P, KT * P], _BF16, kind="ExternalInput")
    bias = nc.dram_tensor("bias", [P, OT], _F32, kind="ExternalInput")
    out = nc.dram_tensor("out", [OUT_F, BR], _F32, kind="ExternalOutput")

    with tile.TileContext(nc) as tc:
        with tc.tile_pool(name="const", bufs=1) as const, \
             tc.tile_pool(name="wslab", bufs=3) as wpool, \
             tc.tile_pool(name="psum", bufs=6, space="PSUM") as pspool, \
             tc.tile_pool(name="yout", bufs=3) as ypool:

            bias_sb = const.tile([P, OT], _F32)
            nc.scalar.dma_start(out=bias_sb[:], in_=bias[:, :])

            # PE warm-up fuel: junk operands with no DMA dependency so
            # matmuls can start right after the engine preamble (~7us)
            # and release the HAM clock throttle early.
            dummy = const.tile([P, P + BR], _BF16)
            nc.vector.memset(dummy[:], 1.0)

            ps_junk = pspool.tile([P, BR], _F32, name="psjunk", tag="ps")

            def junk_mms(n):
                for i in range(n):
                    nc.tensor.matmul(ps_junk[:], lhsT=dummy[:, 0:P],
                                     rhs=dummy[:, P:P + BR],
                                     start=(i == 0), stop=(i == n - 1))

            # ---- x split across scalar HWDGE (k0-15) + gpsimd (k16-31)
            x_sb = const.tile([P, KT * BR], _BF16)
            for it0, nit in XCHUNKS_A:
                nc.scalar.dma_start(
                    out=x_sb[:, it0 * BR:(it0 + nit) * BR],
                    in_=xt[:, it0 * BR:(it0 + nit) * BR])
            for it0, nit in XCHUNKS_B:
                nc.gpsimd.dma_start(
                    out=x_sb[:, it0 * BR:(it0 + nit) * BR],
                    in_=xt[:, it0 * BR:(it0 + nit) * BR])

            # ---- W slabs for the prologue o-tiles (pieced for 0 and 1)
            w_pre = []
            for ot in range(NPRE):
                w_sb = wpool.tile([P, KT * P], _BF16, name=f"w{ot}", tag="w")
                for k0, nk in WPIECES.get(ot, [(0, KT)]):
                    nc.sync.dma_start(
                        out=w_sb[:, k0 * P:(k0 + nk) * P],
                        in_=wt[ot * P:(ot + 1) * P, k0 * P:(k0 + nk) * P])
                w_pre.append(w_sb)

            ps_pre = [pspool.tile([P, BR], _F32, name=f"psp{ot}", tag="ps")
                      for ot in range(NPRE)]

            def mm(ot, it, ps=None, w=None):
                ps = ps if ps is not None else ps_pre[ot]
                w = w if w is not None else w_pre[ot]
                nc.tensor.matmul(
                    ps[:],
                    lhsT=w[:, it * P:(it + 1) * P],
                    rhs=x_sb[:, it * BR:(it + 1) * BR],
                    start=(it == 0), stop=(it == KT - 1))

            def evict(ot, ps):
                y_sb = ypool.tile([P, BR], _F32, name=f"y{ot}", tag="y")
                nc.scalar.activation(y_sb[:], ps[:],
                                     mybir.ActivationFunctionType.Identity,
                                     bias=bias_sb[:, ot:ot + 1])
                nc.scalar.dma_start(out=out[ot * P:(ot + 1) * P, :],
                                    in_=y_sb[:])

            # ---- prologue: o-tiles 0..2 chunk-major in x-arrival order,
            # junk fillers where x/W delivery outpaces the PE.
            junk_mms(5)                      # ~7.0-9.1us, warms HAM
            mm(0, 0); mm(0, 1)
            junk_mms(2)
            mm(0, 2); mm(0, 3)
            junk_mms(3)
            for it in range(4, 8): mm(0, it)
            junk_mms(2)
            for it in range(0, 4): mm(1, it)
            for it in range(8, 12): mm(0, it)
            junk_mms(2)
            for it in range(4, 8): mm(1, it)
            for it in range(12, 16): mm(0, it)
            for it in range(8, 16): mm(1, it)
            for it in range(16, 24): mm(0, it)
            for it in range(16, 24): mm(1, it)
            for it in range(0, 16): mm(2, it)
            for it in range(24, 32): mm(0, it)
            evict(0, ps_pre[0])
            for it in range(24, 32): mm(1, it)
            evict(1, ps_pre[1])
            for it in range(16, 32): mm(2, it)
            evict(2, ps_pre[2])

            # ---- steady state over o-tiles 3..30
            for ot in range(NPRE, OT - 1):
                w_sb = wpool.tile([P, KT * P], _BF16, name=f"w{ot}", tag="w")
                nc.sync.dma_start(out=w_sb[:],
                                  in_=wt[ot * P:(ot + 1) * P, :])
                ps = pspool.tile([P, BR], _F32, name=f"ps{ot}", tag="ps")
                for it in range(KT):
                    mm(ot, it, ps=ps, w=w_sb)
                evict(ot, ps)

            # ---- last o-tile: two half-width accumulation groups so the
            # final eviction + output DMA is half-size and overlapped.
            ot = OT - 1
            w_sb = wpool.tile([P, KT * P], _BF16, name=f"w{ot}", tag="w")
            nc.sync.dma_start(out=w_sb[:], in_=wt[ot * P:(ot + 1) * P, :])
            HB = BR // 2
            ps_a = pspool.tile([P, BR], _F32, name="ps31a", tag="ps")
            ps_b = pspool.tile([P, BR], _F32, name="ps31b", tag="ps")
            for it in range(KT):
                nc.tensor.matmul(
                    ps_a[:, 0:HB],
                    lhsT=w_sb[:, it * P:(it + 1) * P],
                    rhs=x_sb[:, it * BR:it * BR + HB],
                    start=(it == 0), stop=(it == KT - 1))
            y_a = ypool.tile([P, HB], _F32, name="y31a", tag="y")
            nc.scalar.activation(y_a[:], ps_a[:, 0:HB],
                                 mybir.ActivationFunctionType.Identity,
                                 bias=bias_sb[:, ot:ot + 1])
            nc.scalar.dma_start(out=out[ot * P:(ot + 1) * P, 0:HB],
                                in_=y_a[:])
            for it in range(KT):
                nc.tensor.matmul(
                    ps_b[:, 0:HB],
                    lhsT=w_sb[:, it * P:(it + 1) * P],
                    rhs=x_sb[:, it * BR + HB:(it + 1) * BR],
                    start=(it == 0), stop=(it == KT - 1))
            # final eviction: two 128-col pieces on two queues in parallel
            y_b = ypool.tile([P, HB], _F32, name="y31b", tag="y")
            QB = HB // 2
            nc.scalar.activation(y_b[:, 0:QB], ps_b[:, 0:QB],
                                 mybir.ActivationFunctionType.Identity,
                                 bias=bias_sb[:, ot:ot + 1])
            nc.sync.dma_start(out=out[ot * P:(ot + 1) * P, HB:HB + QB],
                              in_=y_b[:, 0:QB])
            nc.scalar.activation(y_b[:, QB:HB], ps_b[:, QB:HB],
                                 mybir.ActivationFunctionType.Identity,
                                 bias=bias_sb[:, ot:ot + 1])
            nc.scalar.dma_start(out=out[ot * P:(ot + 1) * P, HB + QB:BR],
                                in_=y_b[:, QB:HB])

    nc.compile()
    return nc


def _get_nc():
    global _compiled_nc
    if _compiled_nc is None:
        _compiled_nc = _build()
    return _compiled_nc


def _prep_inputs(inputs):
    x = np.ascontiguousarray(np.asarray(inputs["x"], dtype=np.float32))
    w = np.ascontiguousarray(np.asarray(inputs["weight"], dtype=np.float32))
    b = np.ascontiguousarray(np.asarray(inputs["bias"], dtype=np.float32))

    # x tiles per batch group r: [p, it*BR + b] = x_r[b, it*128 + p]
    xts = []
    for r in range(R):
        xs = x[r * BR:(r + 1) * BR, :]                      # [BR, IN_F]
        xt = xs.T.reshape(KT, P, BR).transpose(1, 0, 2)     # [P, KT, BR]
        xts.append(np.ascontiguousarray(
            xt.astype(_BF16_NP).reshape(P, KT * BR)))

    # W tiles: [ot*128 + p, it*128 + o2] = w[ot*128 + o2, it*128 + p]
    wtt = w.T.reshape(KT, P, OT, P).transpose(2, 1, 0, 3)   # [OT,P,KT,P]
    wt = np.ascontiguousarray(wtt.astype(_BF16_NP).reshape(OT * P, KT * P))
    bias_t = np.ascontiguousarray(b.reshape(OT, P).T)       # [P, OT]

    return [{"xt": xts[r], "wt": wt, "bias": bias_t} for r in range(R)]


def _run(inputs, trace=False, trace_cores=None):
    nc = _get_nc()
    in_maps = _prep_inputs(inputs)
    res = run_bass_kernel_spmd(nc, in_maps, core_ids=list(range(N_CORES)),
                               trace=trace, trace_cores=trace_cores)
    y = np.empty((BATCH, OUT_F), dtype=np.float32)
    for r in range(R):
        y[r * BR:(r + 1) * BR, :] = res.results[r]["out"].T
    return y, res


def kernel(**inputs):
    y, _ = _run(inputs)
    return y


# revision 3
# speedup vs baseline: 1.0192x; 1.0192x over previous
"""Trainium2 Bass kernel for y = x @ W^T + b  (4096x4096 @ 4096x4096 + 4096).

Sharding: data-parallel over batch, R=8 groups. Core r computes
yT_r = W @ x_r^T + b[:, None]  ([4096, 512], output transposed) and the
host reassembles y. No collectives.

All layout work happens on the host: x and W are transposed, tiled to
the exact SBUF layout, and cast to bf16 in numpy. The device kernel is
nothing but back-to-back bf16 matmuls (fp32 PSUM accumulate).

v3 schedule (ntff-trace-driven; v1 measured 248.7us, v2's multi-queue x
regressed to 253us because SDMA round-robins queues at packet
granularity and the late-k x transfer starved the critical k0-7 bytes):
  - One queue per stream, bytes in exactly PE-consumption order:
    x k-ascending on the gpsimd ring (~193 GB/s share), W on the sync
    HWDGE queue (~150 GB/s share), bias + outs on the scalar queue.
  - W for the prologue is delivered as k-group pieces interleaved
    across o-tiles 0-2 (the PE's consumption order), with o-tile 3's
    slab trickled in as a 4th stream so the first steady tile has no
    delivery gap.
  - A junk-matmul burst (memset-gated, no DMA dependency) keeps the PE
    busy from ~7us (right after the fixed ~6.6us framework preamble)
    and releases the HAM clock throttle before real matmuls start at
    ~11.3us (first x k-tile arrival; v1 started real work at 22.5us).
  - Prologue: k-major over o0-o2 in x-arrival order, fine-grained x
    chunks so per-chunk matmul work (~1.3us) matches chunk transfer
    time and the PE never idles into a HAM re-throttle.
  - Steady state per o-tile: 1MB W slab DMA (5-deep pool), 32
    k-matmuls (N=512) into one PSUM bank, ScalarE eviction fused with
    bias add, out DMA on the scalar queue (2KB rows, line rate).

PE roofline: 1024 MM x 512 cols @ ~2.4 GHz + issue = ~218-221 us.
"""

import os
import sys

for _p in ("/opt/trn_rl_repo", "/opt/pypackages"):
    if _p not in sys.path and os.path.isdir(_p):
        sys.path.append(_p)

import numpy as np
import ml_dtypes

import concourse.bass as bass
import concourse.tile as tile
from concourse import bacc, mybir
from concourse.bass_utils import run_bass_kernel_spmd

N_CORES = 8
R = 8                          # batch groups
BATCH = 4096
IN_F = 4096
OUT_F = 4096
P = 128
BR = BATCH // R                # 512 batch rows per core
KT = IN_F // P                 # 32 contraction tiles
OT = OUT_F // P                # 32 output-feature tiles per core

# x chunks (k-tile start, len), all on the gpsimd ring in k order.
XCHUNKS = [(0, 1), (1, 1), (2, 2), (4, 2), (6, 2), (8, 4), (12, 4),
           (16, 4), (20, 4), (24, 4), (28, 4)]
# W k-groups for the interleaved prologue delivery.
WGROUPS = [(0, 4), (4, 4), (8, 8), (16, 8), (24, 8)]
NPRE = 3                       # o-tiles interleaved k-major in prologue
NJUNK0 = 10                    # initial junk burst (~4.3us cold)

_F32 = mybir.dt.float32
_BF16 = mybir.dt.bfloat16
_BF16_NP = ml_dtypes.bfloat16

_compiled_nc = None


def _build():
    nc = bacc.Bacc("TRN2", target_bir_lowering=False, debug=False,
                   num_devices=N_CORES)

    # Host-pretiled layouts (see _prep_inputs):
    #   xt[p, it*BR + b]            = x_r[b, it*128 + p]            (bf16)
    #   wt[ot*128 + p, it*128 + o2] = w[ot*128 + o2, it*128 + p]    (bf16)
    #   bias_t[p, ot]               = b[ot*128 + p]                 (f32)
    xt = nc.dram_tensor("xt", [P, KT * BR], _BF16, kind="ExternalInput")
    wt = nc.dram_tensor("wt", [OT * P, KT * P], _BF16, kind="ExternalInput")
    bias = nc.dram_tensor("bias", [P, OT], _F32, kind="ExternalInput")
    out = nc.dram_tensor("out", [OUT_F, BR], _F32, kind="ExternalOutput")

    with tile.TileContext(nc) as tc:
        with tc.tile_pool(name="const", bufs=1) as const, \
             tc.tile_pool(name="wslab", bufs=5) as wpool, \
             tc.tile_pool(name="psum", bufs=6, space="PSUM") as pspool, \
             tc.tile_pool(name="yout", bufs=3) as ypool:

            bias_sb = const.tile([P, OT], _F32)
            nc.scalar.dma_start(out=bias_sb[:], in_=bias[:, :])

            # PE warm-up fuel: junk operands with no DMA dependency so
            # matmuls start right after the engine preamble (~7us) and
            # release the HAM clock throttle before real work arrives.
            dummy = const.tile([P, P + BR], _BF16)
            nc.vector.memset(dummy[:], 1.0)

            ps_junk = pspool.tile([P, BR], _F32, name="psjunk", tag="ps")

            def junk_mms(n):
                for i in range(n):
                    nc.tensor.matmul(ps_junk[:], lhsT=dummy[:, 0:P],
                                     rhs=dummy[:, P:P + BR],
                                     start=(i == 0), stop=(i == n - 1))

            # ---- x on the gpsimd ring, strictly k-ascending
            x_sb = const.tile([P, KT * BR], _BF16)
            for it0, nit in XCHUNKS:
                nc.gpsimd.dma_start(
                    out=x_sb[:, it0 * BR:(it0 + nit) * BR],
                    in_=xt[:, it0 * BR:(it0 + nit) * BR])

            # ---- W: prologue slabs 0-2 as k-groups interleaved in PE
            # consumption order, o3 trickled behind them, o4 prefetched.
            w_pre = [wpool.tile([P, KT * P], _BF16, name=f"w{ot}", tag="w")
                     for ot in range(NPRE + 2)]

            def wdma(ot, k0, nk):
                nc.sync.dma_start(
                    out=w_pre[ot][:, k0 * P:(k0 + nk) * P],
                    in_=wt[ot * P:(ot + 1) * P, k0 * P:(k0 + nk) * P])

            for gi, (k0, nk) in enumerate(WGROUPS):
                for ot in range(NPRE):
                    wdma(ot, k0, nk)
                if gi >= 1:                      # o3 rides along behind
                    wdma(NPRE, WGROUPS[gi - 1][0], WGROUPS[gi - 1][1])
            wdma(NPRE, WGROUPS[-1][0], WGROUPS[-1][1])
            wdma(NPRE + 1, 0, KT)                # o4 full slab

            ps_pre = [pspool.tile([P, BR], _F32, name=f"psp{ot}", tag="ps")
                      for ot in range(NPRE + 1)]

            def mm(ot, it, ps=None, w=None):
                ps = ps if ps is not None else ps_pre[ot]
                w = w if w is not None else w_pre[ot]
                nc.tensor.matmul(
                    ps[:],
                    lhsT=w[:, it * P:(it + 1) * P],
                    rhs=x_sb[:, it * BR:(it + 1) * BR],
                    start=(it == 0), stop=(it == KT - 1))

            def evict(ot, ps):
                y_sb = ypool.tile([P, BR], _F32, name=f"y{ot}", tag="y")
                nc.scalar.activation(y_sb[:], ps[:],
                                     mybir.ActivationFunctionType.Identity,
                                     bias=bias_sb[:, ot:ot + 1])
                nc.scalar.dma_start(out=out[ot * P:(ot + 1) * P, :],
                                    in_=y_sb[:])

            # ---- prologue: k-major over o0-o2 in x-arrival order
            junk_mms(NJUNK0)
            for ci, (it0, nit) in enumerate(XCHUNKS):
                for it in range(it0, it0 + nit):
                    for ot in range(NPRE):
                        mm(ot, it)
                if ci in (0, 1):
                    junk_mms(1)
            for ot in range(NPRE):
                evict(ot, ps_pre[ot])

            # o3: W already streamed with the prologue pieces
            for it in range(KT):
                mm(NPRE, it)
            evict(NPRE, ps_pre[NPRE])

            # ---- steady state over o-tiles 4..31 (o4's W prefetched)
            for ot in range(NPRE + 1, OT):
                if ot == NPRE + 1:
                    w_sb = w_pre[NPRE + 1]
                else:
                    w_sb = wpool.tile([P, KT * P], _BF16,
                                      name=f"w{ot}", tag="w")
                    nc.sync.dma_start(out=w_sb[:],
                                      in_=wt[ot * P:(ot + 1) * P, :])
                ps = pspool.tile([P, BR], _F32, name=f"ps{ot}", tag="ps")
                for it in range(KT):
                    mm(ot, it, ps=ps, w=w_sb)
                evict(ot, ps)

    nc.compile()
    return nc


def _get_nc():
    global _compiled_nc
    if _compiled_nc is None:
        _compiled_nc = _build()
    return _compiled_nc


def _prep_inputs(inputs):
    x = np.ascontiguousarray(np.asarray(inputs["x"], dtype=np.float32))
    w = np.ascontiguousarray(np.asarray(inputs["weight"], dtype=np.float32))
    b = np.ascontiguousarray(np.asarray(inputs["bias"], dtype=np.float32))

    # x tiles per batch group r: [p, it*BR + b] = x_r[b, it*128 + p]
    xts = []
    for r in range(R):
        xs = x[r * BR:(r + 1) * BR, :]                      # [BR, IN_F]
        xt = xs.T.reshape(KT, P, BR).transpose(1, 0, 2)     # [P, KT, BR]
        xts.append(np.ascontiguousarray(
            xt.astype(_BF16_NP).reshape(P, KT * BR)))

    # W tiles: [ot*128 + p, it*128 + o2] = w[ot*128 + o2, it*128 + p]
    wtt = w.T.reshape(KT, P, OT, P).transpose(2, 1, 0, 3)   # [OT,P,KT,P]
    wt = np.ascontiguousarray(wtt.astype(_BF16_NP).reshape(OT * P, KT * P))
    bias_t = np.ascontiguousarray(b.reshape(OT, P).T)       # [P, OT]

    return [{"xt": xts[r], "wt": wt, "bias": bias_t} for r in range(R)]


def _run(inputs, trace=False, trace_cores=None):
    nc = _get_nc()
    in_maps = _prep_inputs(inputs)
    res = run_bass_kernel_spmd(nc, in_maps, core_ids=list(range(N_CORES)),
                               trace=trace, trace_cores=trace_cores)
    y = np.empty((BATCH, OUT_F), dtype=np.float32)
    for r in range(R):
        y[r * BR:(r + 1) * BR, :] = res.results[r]["out"].T
    return y, res


def kernel(**inputs):
    y, _ = _run(inputs)
    return y
